# revision 1
# baseline (speedup 1.0000x reference)
"""ListMLE loss kernel for Trainium2 (8 NeuronCores, data-parallel over batch).

Math (per batch row, N items):
    ss        = scores sorted by `rankings` (gather)
    e         = exp(ss)
    rev[i]    = sum_{j>=i} e[j]            (reverse cumsum)
    loss_row  = sum_{i=0}^{N-2} [ log(rev[i] + eps) - ss[i] ]
    out       = mean(loss_row)

Device-side strategy per core (2048 rows):
    rev[i] = total - fcs[i-1] where fcs = forward inclusive cumsum of e and
    total = sum(e). So per [128, 1024] tile:
      ACT:  e = Exp(ss), accum_out -> total           (1 pass)
      DVE:  fcs = tensor_tensor_scan(add) over e[:, :N-2] written at cols 1..,
            col 0 zeroed -> log terms for i=0..N-2 become one fused op:
      ACT:  Ln(-1 * fcs + total), accum_out -> per-row sum of log terms
      DVE:  reduce_sum(ss[:, :N-1], negate) -> -(sum of ss terms)
    Per-row partials accumulate in a [128, 2*ntiles] staging tile; one final
    DVE reduce -> [128, 1] per-core partial, summed on host.

The gather itself is done host-side while sharding: TRN2 has no
per-partition-indexed gather primitive (GPSIMD indirect ops share indices
across each 16-partition group; DMA gathers are row-granular), so a device
gather would need 16x replicated GPSIMD passes or per-element DMA
descriptors, both orders of magnitude off the memory roofline.
"""

import sys

if "/opt/trn_rl_repo" not in sys.path:
    sys.path.insert(0, "/opt/trn_rl_repo")

from contextlib import ExitStack

import numpy as np

B, N = 16384, 1024
N_CORES = 8
ROWS_PER_CORE = B // N_CORES
P = 128

_CACHE = {}


def build_program(rows_per_core=ROWS_PER_CORE):
    """Build + compile the per-core Bass program (SPMD across 8 cores)."""
    import concourse.bass as bass  # noqa: F401
    import concourse.tile as tile
    from concourse import bacc, mybir

    f32 = mybir.dt.float32
    Act = mybir.ActivationFunctionType
    Alu = mybir.AluOpType
    X = mybir.AxisListType.X

    n_tiles = rows_per_core // P

    nc = bacc.Bacc(
        "TRN2",
        target_bir_lowering=False,
        debug=False,
        enable_asserts=True,
        num_devices=N_CORES,
    )
    ss_d = nc.dram_tensor("ss", [rows_per_core, N], f32, kind="ExternalInput").ap()
    out_d = nc.dram_tensor("partial", [P, 1], f32, kind="ExternalOutput").ap()

    with tile.TileContext(nc) as tc:
        with ExitStack() as ctx:
            pool = ctx.enter_context(tc.tile_pool(name="work", bufs=3))
            spool = ctx.enter_context(tc.tile_pool(name="small", bufs=1))
            # two staging columns per tile: [sum of log terms, -(sum of ss terms)]
            stage = spool.tile([P, 2 * n_tiles], f32)
            for t in range(n_tiles):
                sst = pool.tile([P, N], f32, tag="ss")
                nc.sync.dma_start(sst[:], ss_d[t * P : (t + 1) * P, :])

                es = pool.tile([P, N], f32, tag="es")
                total = pool.tile([P, 1], f32, tag="total")
                nc.scalar.activation(es[:], sst[:], Act.Exp, accum_out=total[:])

                # fcs col 0 = 0, cols 1..N-2 = inclusive cumsum of e[:, 0:N-2]
                fcs = pool.tile([P, N - 1], f32, tag="fcs")
                nc.gpsimd.memset(fcs[:, 0:1], 0.0)
                nc.vector.tensor_tensor_scan(
                    fcs[:, 1 : N - 1],
                    es[:, 0 : N - 2],
                    es[:, 0 : N - 2],
                    0.0,
                    Alu.add,
                    Alu.bypass,
                )

                # log(total - fcs) for all N-1 loss positions; accum -> stage
                logd = pool.tile([P, N - 1], f32, tag="logd")
                nc.scalar.activation(
                    logd[:],
                    fcs[:],
                    Act.Ln,
                    bias=total[:],
                    scale=-1.0,
                    accum_out=stage[:, 2 * t : 2 * t + 1],
                )
                nc.vector.tensor_reduce(
                    stage[:, 2 * t + 1 : 2 * t + 2],
                    sst[:, 0 : N - 1],
                    axis=X,
                    op=Alu.add,
                    negate=True,
                )
            partial = spool.tile([P, 1], f32)
            nc.vector.tensor_reduce(partial[:], stage[:], axis=X, op=Alu.add)
            nc.sync.dma_start(out_d[:], partial[:])

    nc.compile()
    return nc


def _get_program(rows_per_core=ROWS_PER_CORE):
    if rows_per_core not in _CACHE:
        _CACHE[rows_per_core] = build_program(rows_per_core)
    return _CACHE[rows_per_core]


def kernel(scores: np.ndarray, rankings: np.ndarray) -> np.ndarray:
    from concourse import bass_utils

    scores = np.ascontiguousarray(np.asarray(scores, dtype=np.float32))
    rankings = np.asarray(rankings)
    assert scores.shape == (B, N) and rankings.shape == (B, N)

    # Shard prep: sort each row's scores by its ranking (host gather; see
    # module docstring), then split the batch across the 8 cores.
    ss = np.take_along_axis(scores, rankings, axis=1)
    ss = np.ascontiguousarray(ss, dtype=np.float32)

    nc = _get_program()
    in_maps = [
        {"ss": ss[c * ROWS_PER_CORE : (c + 1) * ROWS_PER_CORE]} for c in range(N_CORES)
    ]
    res = bass_utils.run_bass_kernel_spmd(nc, in_maps, core_ids=list(range(N_CORES)))
    total = sum(float(r["partial"].sum()) for r in res.results)
    return np.float32(total / B)



# revision 2
# speedup vs baseline: 1.0633x; 1.0633x over previous
"""ListMLE loss kernel for Trainium2 (8 NeuronCores, data-parallel over batch).

Math (per batch row, N items):
    ss        = scores sorted by `rankings` (gather, done host-side)
    e         = exp(ss)
    rev[p]    = sum_{q>=p} e[q]            (suffix sum)
    loss_row  = sum_{p=0}^{N-2} [ log(rev[p] + eps) - ss[p] ]
    out       = mean(loss_row)

Identity: ln(rev[N-1]) = ln(e[N-1]) = ss[N-1], so
    loss_row = sum_{p=0}^{N-1} ln(rev[p]) - sum_{p=0}^{N-1} ss[p]
(the p = N-1 terms cancel). The host subtracts the grand ss sum in f64.

Measured-HW cost model (ns/instr): scan ~2.15/step + 120, operand strides
free; packed bf16 tensor_tensor ~0.52/elem + 130; ACT ~0.83/elem + 240,
arbitrary APs free; ACT table load 1283. Design:

  - exp writes even/odd PLANES per tile (a[i]=e[2i], b[i]=e[2i+1]).
  - pair-scan: state = (a_i + state) + b_i back-to-front gives
    G[i] = rev[2i] in 512 steps instead of 1024.
  - odd positions additively: s[i] = rev[2i+1] = G[i+1] + b[i] (shifted
    packed TT add against a boundary-zero column; no cancellation, s > 0).
  - fold pp = G*s (packed TT mult) so ACT's ln covers N/2 columns;
    ln(pp) sums both logs. accum_out -> stage, DMA'd out per group.
  - group 0 is emitted per-tile (4 small DMAs + 4 small exps) so the
    Vector engine starts scanning ~7us earlier; later groups use one
    merged DMA + one [128, 4096] exp.
  - during compile, the activation-table list is filtered so Exp and Ln
    resolve only to the combined natural_log_exp_and_others set: one
    table load total instead of one per exp<->ln alternation.

The gather is done host-side while sharding: TRN2 has no per-partition-
indexed gather primitive, so a device gather would need 16x replicated
GPSIMD passes or per-element DMA descriptors, far off the memory roofline.
"""

import sys

if "/opt/trn_rl_repo" not in sys.path:
    sys.path.insert(0, "/opt/trn_rl_repo")

from contextlib import ExitStack

import numpy as np

B, N = 16384, 1024
N_CORES = 8
ROWS_PER_CORE = B // N_CORES
P = 128
K = 4  # row-tiles merged per group
H = N // 2  # pairs per tile
GW = H + 1  # G columns per tile incl boundary zero

_CACHE = {}


def build_program(rows_per_core=ROWS_PER_CORE):
    """Build + compile the per-core Bass program (SPMD across 8 cores)."""
    import concourse.bass as bass  # noqa: F401
    import concourse.bacc as bacc_mod
    import concourse.tile as tile
    from concourse import bacc, mybir

    f32 = mybir.dt.float32
    bf16 = mybir.dt.bfloat16
    Act = mybir.ActivationFunctionType
    Alu = mybir.AluOpType

    n_tiles = rows_per_core // P
    k = min(K, n_tiles)
    assert n_tiles % k == 0
    n_groups = n_tiles // k
    w = k * N  # group width in columns

    nc = bacc.Bacc(
        "TRN2",
        target_bir_lowering=False,
        debug=False,
        enable_asserts=True,
        num_devices=N_CORES,
    )
    ss_d = nc.dram_tensor("ss", [rows_per_core, N], bf16, kind="ExternalInput").ap()
    out_d = nc.dram_tensor("partial", [P, n_groups], f32, kind="ExternalOutput").ap()

    with tile.TileContext(nc) as tc:
        with ExitStack() as ctx:
            pool = ctx.enter_context(tc.tile_pool(name="work", bufs=3))
            gpool = ctx.enter_context(tc.tile_pool(name="gs", bufs=1))
            spool = ctx.enter_context(tc.tile_pool(name="small", bufs=1))
            stage = spool.tile([P, n_groups], f32)
            # ln output is write-only scratch; reuse is hazard-free (ACT is
            # serial), one buffer for all groups.
            logd = spool.tile([P, k * H], bf16)

            # Two persistent gs buffers, alternated per group. The per-tile G
            # boundary columns are zeroed once: scans never write them, the
            # s-add reads them as G[H] = 0.
            gs_bufs = []
            for i in range(2):
                gbuf = gpool.tile([P, k * GW + 2 * k * H], bf16, tag=f"gs{i}")
                for j in range(k):
                    nc.gpsimd.memset(gbuf[:, j * GW + H : j * GW + H + 1], 0.0)
                gs_bufs.append(gbuf)
            s_off = k * GW  # s region
            p_off = k * GW + k * H  # pp region

            def emit_ln(g, gs):
                nc.scalar.activation(
                    logd[:],
                    gs[:, p_off : p_off + k * H],
                    Act.Ln,
                    accum_out=stage[:, g : g + 1],
                )
                nc.sync.dma_start(
                    out_d[:, g : g + 1], stage[:, g : g + 1]
                )

            prev = None
            for g in range(n_groups):
                sst = pool.tile([P, w], bf16, tag="ss")
                es = pool.tile([P, w], bf16, tag="es")
                gs = gs_bufs[g % 2]

                if g == 0:
                    # ramp: per-tile DMA + exp + scan so DVE starts early
                    for j in range(k):
                        nc.sync.dma_start(
                            sst[:, j * N : (j + 1) * N],
                            ss_d[j * P : (j + 1) * P, :],
                        )
                        nc.scalar.activation(
                            es[:, j * N : (j + 1) * N].rearrange(
                                "p (two i) -> p two i", two=2
                            ),
                            sst[:, j * N : (j + 1) * N].rearrange(
                                "p (i two) -> p two i", two=2
                            ),
                            Act.Exp,
                        )
                        a = es[:, j * N : j * N + H]
                        b = es[:, j * N + H : (j + 1) * N]
                        nc.vector.tensor_tensor_scan(
                            gs[:, j * GW : j * GW + H][:, ::-1],
                            a[:, ::-1],
                            b[:, ::-1],
                            0.0,
                            Alu.add,
                            Alu.add,
                        )
                else:
                    nc.sync.dma_start(
                        sst[:].rearrange("p (j c) -> p j c", j=k),
                        ss_d[g * k * P : (g + 1) * k * P, :].rearrange(
                            "(j p) c -> p j c", p=P
                        ),
                    )
                    nc.scalar.activation(
                        es[:].rearrange("p (j two i) -> p j two i", j=k, two=2),
                        sst[:].rearrange("p (j i two) -> p j two i", j=k, two=2),
                        Act.Exp,
                    )
                    for j in range(k):
                        a = es[:, j * N : j * N + H]
                        b = es[:, j * N + H : (j + 1) * N]
                        nc.vector.tensor_tensor_scan(
                            gs[:, j * GW : j * GW + H][:, ::-1],
                            a[:, ::-1],
                            b[:, ::-1],
                            0.0,
                            Alu.add,
                            Alu.add,
                        )

                # s = G_{i+1} + b  (shifted G view incl boundary zeros)
                gshift = gs[:, : k * GW].rearrange("p (j c) -> p j c", j=k)[
                    :, :, 1:
                ]
                bplanes = es[:].rearrange("p (j two i) -> p j two i", j=k, two=2)[
                    :, :, 1
                ]
                nc.vector.tensor_tensor(
                    gs[:, s_off : s_off + k * H].rearrange(
                        "p (j c) -> p j c", j=k
                    ),
                    gshift,
                    bplanes,
                    Alu.add,
                )
                # pp = G * s
                gview = gs[:, : k * GW].rearrange("p (j c) -> p j c", j=k)[
                    :, :, :H
                ]
                nc.vector.tensor_tensor(
                    gs[:, p_off : p_off + k * H].rearrange(
                        "p (j c) -> p j c", j=k
                    ),
                    gview,
                    gs[:, s_off : s_off + k * H].rearrange(
                        "p (j c) -> p j c", j=k
                    ),
                    Alu.mult,
                )

                # software pipeline: ln of the PREVIOUS group
                if prev is not None:
                    emit_ln(*prev)
                prev = (g, gs)
            emit_ln(*prev)

    # Pin Exp/Ln to the combined activation table set so the scheduler's
    # exp/ln interleaving costs one table load total instead of four.
    orig_tables = bacc_mod.get_activation_tables

    def pinned_tables(arch):
        out = {}
        for name, funcs in orig_tables(arch).items():
            if name != "natural_log_exp_and_others":
                funcs = funcs - {Act.Exp, Act.Ln}
            out[name] = funcs
        return out

    bacc_mod.get_activation_tables = pinned_tables
    try:
        nc.compile()
    finally:
        bacc_mod.get_activation_tables = orig_tables
    return nc


def _get_program(rows_per_core=ROWS_PER_CORE):
    if rows_per_core not in _CACHE:
        _CACHE[rows_per_core] = build_program(rows_per_core)
    return _CACHE[rows_per_core]


def _prep(scores: np.ndarray, rankings: np.ndarray):
    """Host shard prep: gather scores by rankings, grand ss sum, bf16 cast."""
    import ml_dtypes

    scores = np.asarray(scores, dtype=np.float32)
    ss = np.take_along_axis(scores, np.asarray(rankings), axis=1)
    s_total = ss.sum(dtype=np.float64)
    ss_bf = np.ascontiguousarray(ss.astype(ml_dtypes.bfloat16))
    return ss_bf, s_total


def kernel(scores: np.ndarray, rankings: np.ndarray) -> np.ndarray:
    from concourse import bass_utils

    assert scores.shape == (B, N) and rankings.shape == (B, N)
    ss_bf, s_total = _prep(scores, rankings)

    nc = _get_program()
    in_maps = [
        {"ss": ss_bf[c * ROWS_PER_CORE : (c + 1) * ROWS_PER_CORE]}
        for c in range(N_CORES)
    ]
    res = bass_utils.run_bass_kernel_spmd(nc, in_maps, core_ids=list(range(N_CORES)))
    d_total = sum(float(r["partial"].sum(dtype=np.float64)) for r in res.results)
    return np.float32((d_total - s_total) / B)


# revision 3
# speedup vs baseline: 1.0790x; 1.0148x over previous
"""ListMLE loss kernel for Trainium2 (8 NeuronCores, data-parallel over batch).

Math (per batch row, N items):
    ss        = scores sorted by `rankings` (gather, done host-side)
    e         = exp(ss)
    rev[p]    = sum_{q>=p} e[q]            (suffix sum)
    loss_row  = sum_{p=0}^{N-2} [ log(rev[p] + eps) - ss[p] ]
    out       = mean(loss_row)

Identity: ln(rev[N-1]) = ln(e[N-1]) = ss[N-1], so
    loss_row = sum_{p=0}^{N-1} ln(rev[p]) - sum_{p=0}^{N-1} ss[p]
(the p = N-1 terms cancel). The host subtracts the grand ss sum in f64.

Measured-HW cost model (ns/instr): scan ~2.15/step + 120, operand strides
free; packed bf16 tensor_tensor ~0.52/elem + 130; ACT ~0.83/elem + 240,
arbitrary APs free; ACT table load 1283. Design:

  - exp writes even/odd PLANES per tile (a[i]=e[2i], b[i]=e[2i+1]).
  - pair-scan: state = (a_i + state) + b_i back-to-front gives
    G[i] = rev[2i] in 512 steps instead of 1024.
  - odd positions additively: s[i] = rev[2i+1] = G[i+1] + b[i] (shifted
    packed TT add against a boundary-zero column; no cancellation, s > 0).
  - fold pp = G*s (packed TT mult) so ACT's ln covers N/2 columns;
    ln(pp) sums both logs. accum_out -> stage, DMA'd out per group.
  - group 0 is emitted per-tile (4 small DMAs + 4 small exps) so the
    Vector engine starts scanning ~7us earlier; later groups use one
    merged DMA + one [128, 4096] exp.
  - during compile, the activation-table list is filtered so Exp and Ln
    resolve only to the combined natural_log_exp_and_others set: one
    table load total instead of one per exp<->ln alternation.

The gather is done host-side while sharding: TRN2 has no per-partition-
indexed gather primitive, so a device gather would need 16x replicated
GPSIMD passes or per-element DMA descriptors, far off the memory roofline.
"""

import sys

if "/opt/trn_rl_repo" not in sys.path:
    sys.path.insert(0, "/opt/trn_rl_repo")

from contextlib import ExitStack

import numpy as np

B, N = 16384, 1024
N_CORES = 8
ROWS_PER_CORE = B // N_CORES
P = 128
K = 4  # row-tiles merged per group
H = N // 2  # pairs per tile
GW = H + 1  # G columns per tile incl boundary zero

_CACHE = {}


def build_program(rows_per_core=ROWS_PER_CORE):
    """Build + compile the per-core Bass program (SPMD across 8 cores)."""
    import concourse.bass as bass  # noqa: F401
    import concourse.bacc as bacc_mod
    import concourse.tile as tile
    from concourse import bacc, mybir

    f32 = mybir.dt.float32
    bf16 = mybir.dt.bfloat16
    Act = mybir.ActivationFunctionType
    Alu = mybir.AluOpType

    n_tiles = rows_per_core // P
    k = min(K, n_tiles)
    assert n_tiles % k == 0
    n_groups = n_tiles // k
    w = k * N  # group width in columns

    nc = bacc.Bacc(
        "TRN2",
        target_bir_lowering=False,
        debug=False,
        enable_asserts=True,
        num_devices=N_CORES,
    )
    ss_d = nc.dram_tensor("ss", [rows_per_core, N], bf16, kind="ExternalInput").ap()
    out_d = nc.dram_tensor("partial", [P, n_groups + 1], f32, kind="ExternalOutput").ap()

    with tile.TileContext(nc) as tc:
        with ExitStack() as ctx:
            pool = ctx.enter_context(tc.tile_pool(name="work", bufs=3))
            gpool = ctx.enter_context(tc.tile_pool(name="gs", bufs=1))
            spool = ctx.enter_context(tc.tile_pool(name="small", bufs=1))
            stage = spool.tile([P, n_groups + 1], f32)
            # ln output is write-only scratch; reuse is hazard-free (ACT is
            # serial), one buffer for all groups.
            logd = spool.tile([P, k * H], bf16)

            # Two persistent gs buffers, alternated per group. The per-tile G
            # boundary columns are zeroed once: scans never write them, the
            # s-add reads them as G[H] = 0.
            gs_bufs = []
            for i in range(2):
                gbuf = gpool.tile([P, k * GW + 2 * k * H], bf16, tag=f"gs{i}")
                for j in range(k):
                    nc.gpsimd.memset(gbuf[:, j * GW + H : j * GW + H + 1], 0.0)
                gs_bufs.append(gbuf)
            s_off = k * GW  # s region
            p_off = k * GW + k * H  # pp region

            def emit_ln(g, gs):
                if g == 1:
                    gview = gs[:, : k * GW].rearrange("p (j c) -> p j c", j=k)[
                        :, :, :H
                    ]
                    nc.scalar.activation(
                        logd[:], gview, Act.Ln,
                        accum_out=stage[:, g : g + 1],
                    )
                    nc.scalar.activation(
                        logd[:],
                        gs[:, s_off : s_off + k * H],
                        Act.Ln,
                        accum_out=stage[:, n_groups : n_groups + 1],
                    )
                    nc.sync.dma_start(
                        out_d[:, n_groups : n_groups + 1],
                        stage[:, n_groups : n_groups + 1],
                    )
                else:
                    nc.scalar.activation(
                        logd[:],
                        gs[:, p_off : p_off + k * H],
                        Act.Ln,
                        accum_out=stage[:, g : g + 1],
                    )
                nc.sync.dma_start(
                    out_d[:, g : g + 1], stage[:, g : g + 1]
                )

            prev = None
            for g in range(n_groups):
                sst = pool.tile([P, w], bf16, tag="ss")
                es = pool.tile([P, w], bf16, tag="es")
                gs = gs_bufs[g % 2]

                if g == 0:
                    # ramp: per-tile DMA + exp + scan so DVE starts early
                    for j in range(k):
                        nc.sync.dma_start(
                            sst[:, j * N : (j + 1) * N],
                            ss_d[j * P : (j + 1) * P, :],
                        )
                        nc.scalar.activation(
                            es[:, j * N : (j + 1) * N].rearrange(
                                "p (two i) -> p two i", two=2
                            ),
                            sst[:, j * N : (j + 1) * N].rearrange(
                                "p (i two) -> p two i", two=2
                            ),
                            Act.Exp,
                        )
                        a = es[:, j * N : j * N + H]
                        b = es[:, j * N + H : (j + 1) * N]
                        nc.vector.tensor_tensor_scan(
                            gs[:, j * GW : j * GW + H][:, ::-1],
                            a[:, ::-1],
                            b[:, ::-1],
                            0.0,
                            Alu.add,
                            Alu.add,
                        )
                else:
                    nc.sync.dma_start(
                        sst[:].rearrange("p (j c) -> p j c", j=k),
                        ss_d[g * k * P : (g + 1) * k * P, :].rearrange(
                            "(j p) c -> p j c", p=P
                        ),
                    )
                    nc.scalar.activation(
                        es[:].rearrange("p (j two i) -> p j two i", j=k, two=2),
                        sst[:].rearrange("p (j i two) -> p j two i", j=k, two=2),
                        Act.Exp,
                    )
                    for j in range(k):
                        a = es[:, j * N : j * N + H]
                        b = es[:, j * N + H : (j + 1) * N]
                        nc.vector.tensor_tensor_scan(
                            gs[:, j * GW : j * GW + H][:, ::-1],
                            a[:, ::-1],
                            b[:, ::-1],
                            0.0,
                            Alu.add,
                            Alu.add,
                        )

                # s = G_{i+1} + b  (shifted G view incl boundary zeros)
                gshift = gs[:, : k * GW].rearrange("p (j c) -> p j c", j=k)[
                    :, :, 1:
                ]
                bplanes = es[:].rearrange("p (j two i) -> p j two i", j=k, two=2)[
                    :, :, 1
                ]
                nc.vector.tensor_tensor(
                    gs[:, s_off : s_off + k * H].rearrange(
                        "p (j c) -> p j c", j=k
                    ),
                    gshift,
                    bplanes,
                    Alu.add,
                )
                if g != 1:
                    # pp = G * s
                    gview = gs[:, : k * GW].rearrange("p (j c) -> p j c", j=k)[
                        :, :, :H
                    ]
                    nc.vector.tensor_tensor(
                        gs[:, p_off : p_off + k * H].rearrange(
                            "p (j c) -> p j c", j=k
                        ),
                        gview,
                        gs[:, s_off : s_off + k * H].rearrange(
                            "p (j c) -> p j c", j=k
                        ),
                        Alu.mult,
                    )

                # software pipeline: ln of the PREVIOUS group
                if prev is not None:
                    emit_ln(*prev)
                prev = (g, gs)
            emit_ln(*prev)

    # Pin Exp/Ln to the combined activation table set so the scheduler's
    # exp/ln interleaving costs one table load total instead of four.
    orig_tables = bacc_mod.get_activation_tables

    def pinned_tables(arch):
        out = {}
        for name, funcs in orig_tables(arch).items():
            if name != "natural_log_exp_and_others":
                funcs = funcs - {Act.Exp, Act.Ln}
            out[name] = funcs
        return out

    bacc_mod.get_activation_tables = pinned_tables
    try:
        nc.compile()
    finally:
        bacc_mod.get_activation_tables = orig_tables
    return nc


def _get_program(rows_per_core=ROWS_PER_CORE):
    if rows_per_core not in _CACHE:
        _CACHE[rows_per_core] = build_program(rows_per_core)
    return _CACHE[rows_per_core]


def _prep(scores: np.ndarray, rankings: np.ndarray):
    """Host shard prep: gather scores by rankings, grand ss sum, bf16 cast."""
    import ml_dtypes

    scores = np.asarray(scores, dtype=np.float32)
    ss = np.take_along_axis(scores, np.asarray(rankings), axis=1)
    s_total = ss.sum(dtype=np.float64)
    ss_bf = np.ascontiguousarray(ss.astype(ml_dtypes.bfloat16))
    return ss_bf, s_total


def kernel(scores: np.ndarray, rankings: np.ndarray) -> np.ndarray:
    from concourse import bass_utils

    assert scores.shape == (B, N) and rankings.shape == (B, N)
    ss_bf, s_total = _prep(scores, rankings)

    nc = _get_program()
    in_maps = [
        {"ss": ss_bf[c * ROWS_PER_CORE : (c + 1) * ROWS_PER_CORE]}
        for c in range(N_CORES)
    ]
    res = bass_utils.run_bass_kernel_spmd(nc, in_maps, core_ids=list(range(N_CORES)))
    d_total = sum(float(r["partial"].sum(dtype=np.float64)) for r in res.results)
    return np.float32((d_total - s_total) / B)
